# revision 41
# baseline (speedup 1.0000x reference)
"""Trainium2 Bass kernel for nn_LocalFeatureGuided.

Pipeline per image (C=128 on partitions, spatial on free dim):
  BN(eval)+GELU (ACT, fused affine) -> even/odd column split buffers (bf16)
  depthwise 7x7 s2 conv: 49 taps split by output-row ranges across
    PE (diag-matmul bf16, PSUM accum) and DVE (scalar_tensor_tensor FMA)
  tokens: t0=guide, t1..4 = strided views of x (no copies)
  q0 = WqT.T@t0 ; k_m = WkT.T@t_m consumed from PSUM by DVE
    dot-products -> s_m = <q0,k_m>
  softmax over 5 logits per (b,c); v & proj fused:
    out = sum_m (Wv^T diag(a_m) Pw^T)^T @ t_m  (5 accumulating matmuls)
Host-side prep: x downcast to bf16, weights pre-transposed + conv diag
matrices prebuilt, BN affine folded.
Sharding: data-parallel over batch, 2 images per core, 8 cores.
"""

import os
import numpy as np
from contextlib import ExitStack

import concourse.bass as bass
import concourse.tile as tile
from concourse import bacc, mybir
from concourse import bass_utils
from concourse import tile_utils

alu = mybir.AluOpType
actf = mybir.ActivationFunctionType
F32 = mybir.dt.float32
BF16 = mybir.dt.bfloat16
FP8 = mybir.dt.float8e4

B, C, H, W = 16, 128, 128, 128
H2, W2 = H // 2, W // 2
L = H2 * W2            # 4096
NCORES = 8
BPC = B // NCORES      # 2 images per core
EPS = 1e-5
INV_SQRT_C = 1.0 / np.sqrt(128.0)

# ---- tuning knobs ----
NR_DVE = int(os.environ.get("NR_DVE", "8"))  # conv rows [0,NR_DVE) on DVE
SBUF_CAP = 204 * 1024

KSTAGE = int(os.environ.get("KSTAGE", "9"))

KH_LO = [2, 1, 1, 0, 0, 0, 0]
KH_HI = [64, 64, 64, 64, 64, 63, 63]

TAPS = [(kh, kw) for kh in range(7) for kw in range(7)]


def build(nc):
    x_d = nc.dram_tensor("x", (BPC, C, H, W), BF16, kind="ExternalInput").ap()
    bns_d = nc.dram_tensor("bns", (C, 1), F32, kind="ExternalInput").ap()
    bnb_d = nc.dram_tensor("bnb", (C, 1), F32, kind="ExternalInput").ap()
    dww_d = nc.dram_tensor("dww", (C, 49), F32, kind="ExternalInput").ap()
    dwb_d = nc.dram_tensor("dwb", (C, 1), F32, kind="ExternalInput").ap()
    diag_d = nc.dram_tensor("diag", (C, 49 * 128), FP8,
                            kind="ExternalInput").ap()
    csc_d = nc.dram_tensor("cscale", (C, 1), F32, kind="ExternalInput").ap()
    wqT_d = nc.dram_tensor("wqT", (C, C), BF16, kind="ExternalInput").ap()
    wkT_d = nc.dram_tensor("wkT", (C, C), BF16, kind="ExternalInput").ap()
    wv_d = nc.dram_tensor("wv", (C, C), BF16, kind="ExternalInput").ap()
    pwT_d = nc.dram_tensor("pwT", (C, C), F32, kind="ExternalInput").ap()
    bq_d = nc.dram_tensor("bq", (C, 1), F32, kind="ExternalInput").ap()
    bk_d = nc.dram_tensor("bk", (C, 1), F32, kind="ExternalInput").ap()
    cb_d = nc.dram_tensor("cb", (C, 1), F32, kind="ExternalInput").ap()
    out_d = nc.dram_tensor("out", (BPC, C, H2, W2), F32,
                           kind="ExternalOutput").ap()

    with tile.TileContext(nc) as tc, ExitStack() as ctx:
        tp = lambda name, bufs, **kw: ctx.enter_context(
            tc.tile_pool(name=name, bufs=bufs, **kw))

        wpool = tp("weights", 1)       # persistent small weights
        xp = tp("x", 2)
        gp = tp("gelu", 2)
        t0p = tp("t0", 2)
        q0p = tp("q0", 2)
        accp = tp("acc", 2)
        outp = tp("outc", 3)
        vecp = tp("vec", 16)
        emp = tp("em", 2)
        scrp = tp("scr", 2)
        vtp = tp("vt", 10)
        pp512 = tp("pp512", 2, space="PSUM")
        ppk = tp("ppk", 3, space="PSUM")   # [128,1024] = 2 banks each

        # ---------- persistent weights ----------
        def vec_load(src_ap):
            t = vecp.tile([C, 1], F32, tag="v")
            nc.gpsimd.dma_start(t[:], src_ap)
            return t

        bns = vec_load(bns_d)
        bnb = vec_load(bnb_d)
        dwb = vec_load(dwb_d)
        csc = vec_load(csc_d)
        bq = vec_load(bq_d)
        bk = vec_load(bk_d)
        cb = vec_load(cb_d)

        dww = wpool.tile([C, 49], F32)
        nc.gpsimd.dma_start(dww[:], dww_d)
        diag = wpool.tile([C, 49 * 128], FP8)
        wqT = wpool.tile([C, C], BF16)
        nc.gpsimd.dma_start(wqT[:], wqT_d)
        wkT = wpool.tile([C, C], BF16)
        nc.gpsimd.dma_start(wkT[:], wkT_d)
        wv = wpool.tile([C, C], BF16)
        nc.gpsimd.dma_start(wv[:], wv_d)
        pwT = wpool.tile([C, C], F32)
        nc.gpsimd.dma_start(pwT[:], pwT_d)

        # eo buffers allocated up-front; pads are memset once and never
        # overwritten (gelu writes only the interior), so the per-image
        # gelu never waits on a pad memset
        eo_bufs = []
        for _ in range(BPC):
            eo = gp.tile([C, 2, 134, 68], FP8, tag="eo")
            nc.vector.memset(eo[:, :, 0:3], 0.0)
            nc.vector.memset(eo[:, :, 131:134], 0.0)
            nc.vector.memset(eo[:, 0, 3:131, 0:1], 0.0)
            nc.vector.memset(eo[:, 0, 3:131, 65:68], 0.0)
            nc.vector.memset(eo[:, 1, 3:131, 0:2], 0.0)
            nc.vector.memset(eo[:, 1, 3:131, 66:68], 0.0)
            eo_bufs.append(eo)

        # ---------- per image ----------
        for img in range(BPC):
            x_t = xp.tile([C, H * W], BF16)
            xi = x_d[img].rearrange("c h w -> c (h w)")
            # finer leading slices so gelu band 0 starts ASAP (per-queue
            # DMA bandwidth bounds each slice's completion time)
            xsl = [512 * s for s in range(8)] + \
                  [4096 + 1024 * s for s in range(12)] + [H * W]
            for si, (a, b) in enumerate(zip(xsl, xsl[1:])):
                nc.sync.dma_start(x_t[:, a:b], xi[:, a:b])
                if img == 0 and si == 3:
                    # weights arrive behind the first gelu bands' x slices
                    for s in range(8):
                        sl = slice(s * 784, (s + 1) * 784)
                        nc.sync.dma_start(diag[:, sl], diag_d[:, sl])
            x3 = x_t[:].rearrange("c (h w) -> c h w", h=H)

            # BN+GELU into zero-padded even/odd column buffers:
            #   E[3+r, 1+j] = g[r, 2j]   (width 68, cols 0 and 65.. are pad)
            #   O[3+r, 2+j] = g[r, 2j+1] (cols 0,1 and 66.. are pad)
            # rows 0..2 and 131..133 are pad.
            eo = eo_bufs[img]
            for bnd in range(8):  # fine row bands so conv can start early
                r0, r1 = 16 * bnd, 16 * (bnd + 1)
                nc.scalar.activation(
                    eo[:, 0, 3 + r0:3 + r1, 1:65],
                    x3[:, r0:r1, 0::2], actf.Gelu,
                    bias=bnb[:, 0:1], scale=bns[:, 0:1])
                nc.scalar.activation(
                    eo[:, 1, 3 + r0:3 + r1, 2:66],
                    x3[:, r0:r1, 1::2], actf.Gelu,
                    bias=bnb[:, 0:1], scale=bns[:, 0:1])

            oi = out_d[img].rearrange("c h w -> c (h w)")
            t0 = t0p.tile([C, L], BF16)
            q0 = q0p.tile([C, L], BF16)
            q0sums = vecp.tile([C, 8], F32, tag="q0s")

            def g_ap(kh, kw, a, b):
                # full-rect tap read for h2 in [a,b), all w2: row 2*h2+kh,
                # col (pad+u)+w2 in the parity buffer
                e = kw - 3
                par, u = (0, e // 2) if e % 2 == 0 else (1, (e - 1) // 2)
                off = (1 + u) if par == 0 else (2 + u)
                return eo[:, par, kh + 2 * a:kh + 2 * b:2, off:off + 64]

            def diag_ap(kh, kw):
                t = kh * 7 + kw
                return diag[:, t * 128:(t + 1) * 128]

            def q0_mm(ch):
                ps = pp512.tile([C, 512], F32)
                nc.tensor.matmul(ps[:], wqT[:],
                                 t0[:, ch * 512:(ch + 1) * 512],
                                 start=True, stop=True)
                nc.scalar.activation(q0[:, ch * 512:(ch + 1) * 512], ps[:],
                                     actf.Identity, bias=bq[:, 0:1],
                                     accum_out=q0sums[:, ch:ch + 1])

            # --- conv: DVE part (rows [0, NR_DVE)) — two independent
            # accumulation chains so back-to-back FMAs pipeline without
            # waiting on the previous op's writeback ack ---
            if NR_DVE:
                acc0 = accp.tile([C, NR_DVE, 64], F32, tag="a0")
                acc1 = accp.tile([C, NR_DVE, 64], F32, tag="a1")
                accs = [acc0, acc1]
                for i, (kh, kw) in enumerate(TAPS):
                    w_s = dww[:, kh * 7 + kw:kh * 7 + kw + 1]
                    acc = accs[i % 2]
                    if i < 2:
                        nc.vector.tensor_scalar_mul(
                            acc[:], g_ap(kh, kw, 0, NR_DVE), w_s)
                    else:
                        nc.vector.scalar_tensor_tensor(
                            acc[:], g_ap(kh, kw, 0, NR_DVE), w_s, acc[:],
                            alu.mult, alu.add)
                nc.vector.tensor_tensor(accs[0][:], accs[0][:], accs[1][:],
                                        alu.add)
                nc.scalar.activation(t0[:, :NR_DVE * 64],
                                     accs[0][:].rearrange("c h w -> c (h w)"),
                                     actf.Identity, bias=dwb[:, 0:1])

            # token access patterns (m=0 guide, m>=1 raw-x 2x2 windows)
            def tok_ap(m, c0, c1):
                if m == 0:
                    return t0[:, c0:c1]
                p, q = (m - 1) // 2, (m - 1) % 2
                assert c0 % 64 == 0 and c1 % 64 == 0
                return x3[:, p::2, q::2][:, c0 // 64:c1 // 64, :]

            # dots[:, m*4+hf] = <q0_hf, k_m_hf>; summed in softmax reduce
            dots = vecp.tile([C, 20], F32, tag="dots")

            def k_mm(m, hf):
                # one [C,1024] dot op per (m, ch-pair): 2 matmuls + 1 stt
                kp = ppk.tile([C, 1024], F32, tag="kp")
                for j in range(2):
                    c0 = hf * 1024 + j * 512
                    nc.tensor.matmul(kp[:, j * 512:(j + 1) * 512], wkT[:],
                                     tok_ap(m, c0, c0 + 512),
                                     start=True, stop=True)
                scr = scrp.tile([C, 1024], F32, tag="s")
                nc.vector.scalar_tensor_tensor(
                    scr[:], q0[:, hf * 1024:(hf + 1) * 1024], 1.0, kp[:],
                    alu.mult, alu.mult,
                    accum_out=dots[:, m * 4 + hf:m * 4 + hf + 1])

            # --- conv: PE part (h2 rows [NR_DVE, 64)), fp8 DoubleRow.
            # The even/odd parity planes of eo let tap pairs (kw=2j+1,
            # kw=2j) share one rhs AP with the parity axis as the pair dim:
            # out = diagE.T @ eo[:,0,...] + diagO.T @ eo[:,1,...].
            # 21 pairs + 7 singles (kw=6) instead of 49 matmuls per chunk.
            chunks = [(r, min(r + 8, 64)) for r in range(NR_DVE, 64, 8)]
            for ci, (r0, r1) in enumerate(chunks):
                nrw = r1 - r0
                ps = pp512.tile([C, nrw * 64], F32)
                kq = {  # weave (m, hf) dots; q0 chs land at end of iter ch
                    4: [(0, 1), (1, 1), (2, 1)],
                    5: [(3, 1), (4, 1), (0, 0), (1, 0)],
                    6: [(0, 2), (1, 2), (2, 2), (2, 0), (3, 2), (4, 2)],
                }.get(ci, [])
                for p in range(21):  # pair index: kh = p // 3, j = p % 3
                    kh, j = p // 3, p % 3
                    lhsT = diag[:, p * 256:(p + 1) * 256].rearrange(
                        "c (two f) -> c two f", two=2)
                    rhs = eo[:, :, kh + 2 * r0:kh + 2 * r1:2, j:j + 64]
                    nc.tensor.matmul(ps[:], lhsT, rhs, start=(p == 0),
                                     stop=False,
                                     perf_mode=mybir.MatmulPerfMode.DoubleRow)
                    if p in (3, 7, 11, 15, 19) and kq:
                        k_mm(*kq.pop(0))  # smooth kp flow into DVE
                for pi, kh in enumerate((0, 1, 4)):
                    # kw=6 taps paired (kh, kh+2) via an overlapping-stride
                    # pair dim hand-built on the row axis of the odd plane
                    lhsT = diag[:, (42 + 2 * pi) * 128:
                                (44 + 2 * pi) * 128].rearrange(
                        "c (two f) -> c two f", two=2)
                    rhs = eo[:, 1, kh + 2 * r0:kh + 2 * r1:2, 3:67]
                    rhs = rhs.unsqueeze(1)
                    rhs.ap[1] = [136, 2]  # second plane = rows +2 (tap kh+2)
                    nc.tensor.matmul(ps[:], lhsT, rhs, start=False,
                                     stop=False,
                                     perf_mode=mybir.MatmulPerfMode.DoubleRow)
                nc.tensor.matmul(  # lone tap (kh=5, kw=6)
                    ps[:], diag[:, 48 * 128:49 * 128],
                    eo[:, 1, 5 + 2 * r0:5 + 2 * r1:2, 3:67],
                    start=False, stop=True)
                while kq:
                    k_mm(*kq.pop(0))
                nc.scalar.activation(t0[:, r0 * 64:r1 * 64], ps[:],
                                     actf.Identity, bias=dwb[:, 0:1],
                                     scale=csc[:, 0:1])
                if ci >= 1:
                    q0_mm(ci)  # q0 over the t0 chunk copied last iteration
                if ci == 4:
                    q0_mm(0)  # DVE-region t0 landed during early chunks
            q0_mm(7)
            for mh in ((3, 0), (0, 3), (4, 0),
                       (1, 3), (2, 3), (3, 3), (4, 3)):
                k_mm(*mh)

            if KSTAGE == 2:
                oc = outp.tile([C, L], F32, tag="dbg")
                nc.scalar.copy(oc[:], t0[:])
                nc.sync.dma_start(oi, oc[:])
                continue

            # --- softmax over 5 logits (no max-sub: logits are O(1)) ---
            s5 = vecp.tile([C, 5], F32, tag="s5")
            nc.vector.tensor_reduce(
                s5[:], dots[:].rearrange("c (m h) -> c m h", m=5),
                mybir.AxisListType.X, alu.add)
            q0s = vecp.tile([C, 1], F32, tag="v")
            nc.vector.tensor_reduce(q0s[:], q0sums[:], mybir.AxisListType.X,
                                    alu.add)
            bkqs = vecp.tile([C, 1], F32, tag="v")
            nc.vector.scalar_tensor_tensor(bkqs[:], bk[:], INV_SQRT_C, q0s[:],
                                           alu.mult, alu.mult)
            e5 = vecp.tile([C, 5], F32, tag="s5")
            nc.scalar.activation(e5[:], s5[:], actf.Exp, bias=bkqs[:, 0:1],
                                 scale=INV_SQRT_C)
            ssum = vecp.tile([C, 1], F32, tag="v")
            nc.vector.tensor_reduce(ssum[:], e5[:], mybir.AxisListType.X,
                                    alu.add)
            sinv = vecp.tile([C, 1], F32, tag="v")
            nc.vector.reciprocal(sinv[:], ssum[:])
            a5 = vecp.tile([C, 5], F32, tag="s5")
            nc.vector.tensor_scalar_mul(a5[:], e5[:], sinv[:, 0:1])

            # --- fused v+proj: lhsT_m = Wv^T diag(a_m) Pw^T ---
            # em_all[:, m*128+d] = pwT[:, d] * a5[:, m] in one broadcast op,
            # then two matmuls + two copies produce all five vt blocks
            em_all = emp.tile([C, 5, C], BF16, tag="em")
            nc.vector.tensor_tensor(
                em_all[:],
                pwT[:].rearrange("c (m d) -> c m d", m=1).broadcast_to(
                    (C, 5, C)),
                a5[:].rearrange("c (m o) -> c m o", o=1).broadcast_to(
                    (C, 5, C)),
                alu.mult)
            emf = em_all[:].rearrange("c m d -> c (m d)")
            vt_all = vtp.tile([C, 5 * C], BF16, tag="vt")
            for lo, hi in ((0, 4), (4, 5)):
                vp = ppk.tile([C, 512], F32, tag="kp")
                nc.tensor.matmul(vp[:, :(hi - lo) * C], wv[:],
                                 emf[:, lo * C:hi * C], start=True, stop=True)
                nc.scalar.copy(vt_all[:, lo * C:hi * C],
                               vp[:, :(hi - lo) * C])

            for ch in range(8):
                ps = pp512.tile([C, 512], F32)
                for m in range(5):
                    nc.tensor.matmul(
                        ps[:], vt_all[:, m * C:(m + 1) * C],
                        tok_ap(m, ch * 512, (ch + 1) * 512),
                        start=(m == 0), stop=(m == 4))
                oc = outp.tile([C, 512], F32, tag="oc")
                nc.scalar.activation(oc[:], ps[:], actf.Identity,
                                     bias=cb[:, 0:1])
                nc.sync.dma_start(oi[:, ch * 512:(ch + 1) * 512], oc[:])
    return nc


_CACHE = {}


def _get_nc():
    if "nc" not in _CACHE:
        tile_utils.max_sbuf_usage = SBUF_CAP
        nc = bacc.Bacc("TRN2", target_bir_lowering=False, debug=False,
                       num_devices=NCORES)
        build(nc)
        nc.compile()
        _CACHE["nc"] = nc
    return _CACHE["nc"]


def _in_maps(x, bn_gamma, bn_beta, bn_mean, bn_var, dw_w, dw_b, qkv_w, qkv_b,
             proj_w, proj_b):
    import ml_dtypes
    bf16 = ml_dtypes.bfloat16
    f32 = np.float32
    bn_gamma = np.asarray(bn_gamma, f32)
    bn_beta = np.asarray(bn_beta, f32)
    bn_mean = np.asarray(bn_mean, f32)
    bn_var = np.asarray(bn_var, f32)
    dw_w = np.asarray(dw_w, f32).reshape(C, 49)
    dw_b = np.asarray(dw_b, f32)
    qkv_w = np.asarray(qkv_w, f32)
    qkv_b = np.asarray(qkv_b, f32)
    proj_w = np.asarray(proj_w, f32)
    proj_b = np.asarray(proj_b, f32)

    bns = bn_gamma / np.sqrt(bn_var + np.float32(EPS))
    bnb = bn_beta - bn_mean * bns

    # fp8 conv weights, pre-scaled per channel by a power of two so they
    # sit in e4m3's normal range; the t0 copy descales via ACT's scale.
    wmax = np.abs(dw_w).max(axis=1)
    S = np.exp2(np.floor(np.log2(128.0 / np.maximum(wmax, 1e-30)))).astype(f32)
    cscale = (1.0 / S).reshape(C, 1)
    # block order: 21 pairs [(kh, 2j+1) even-plane, (kh, 2j) odd-plane],
    # then kw=6 row-pairs [(kh,6),(kh+2,6)] for kh in (0,1,4), then (5,6)
    diag = np.zeros((C, 49 * 128), f32)
    blocks = []
    for kh in range(7):
        for j in range(3):
            blocks.append((kh, 2 * j + 1))
            blocks.append((kh, 2 * j))
    for kh in (0, 1, 4):
        blocks.append((kh, 6))
        blocks.append((kh + 2, 6))
    blocks.append((5, 6))
    for bi, (kh, kw) in enumerate(blocks):
        diag[np.arange(C), bi * 128 + np.arange(C)] = dw_w[:, kh * 7 + kw] * S

    bv = qkv_b[2 * C:3 * C]
    fp8 = ml_dtypes.float8_e4m3fn
    shared = {
        "bns": bns.reshape(C, 1),
        "bnb": bnb.reshape(C, 1),
        "dww": np.ascontiguousarray(dw_w),
        "dwb": dw_b.reshape(C, 1),
        "diag": diag.astype(fp8),
        "cscale": cscale,
        "wqT": np.ascontiguousarray(qkv_w[0:C].T).astype(bf16),
        "wkT": np.ascontiguousarray(qkv_w[C:2 * C].T).astype(bf16),
        "wv": np.ascontiguousarray(qkv_w[2 * C:3 * C]).astype(bf16),
        "pwT": np.ascontiguousarray(proj_w.T),
        "bq": qkv_b[0:C].reshape(C, 1),
        "bk": qkv_b[C:2 * C].reshape(C, 1),
        "cb": (proj_w @ bv + proj_b).reshape(C, 1),
    }
    xf = np.ascontiguousarray(np.asarray(x, f32)).astype(bf16)
    return [dict(shared, x=xf[i * BPC:(i + 1) * BPC]) for i in range(NCORES)]


def kernel(**inputs):
    nc = _get_nc()
    res = bass_utils.run_bass_kernel_spmd(nc, _in_maps(**inputs),
                                          core_ids=list(range(NCORES)))
    return np.concatenate([r["out"] for r in res.results], axis=0)


# revision 44
# speedup vs baseline: 1.0283x; 1.0283x over previous
"""Trainium2 Bass kernel for nn_LocalFeatureGuided.

Pipeline per image (C=128 on partitions, spatial on free dim):
  BN(eval)+GELU (ACT, fused affine) -> even/odd column split buffers (bf16)
  depthwise 7x7 s2 conv: 49 taps split by output-row ranges across
    PE (diag-matmul bf16, PSUM accum) and DVE (scalar_tensor_tensor FMA)
  tokens: t0=guide, t1..4 = strided views of x (no copies)
  q0 = WqT.T@t0 ; k_m = WkT.T@t_m consumed from PSUM by DVE
    dot-products -> s_m = <q0,k_m>
  softmax over 5 logits per (b,c); v & proj fused:
    out = sum_m (Wv^T diag(a_m) Pw^T)^T @ t_m  (5 accumulating matmuls)
Host-side prep: x downcast to bf16, weights pre-transposed + conv diag
matrices prebuilt, BN affine folded.
Sharding: data-parallel over batch, 2 images per core, 8 cores.
"""

import os
import numpy as np
from contextlib import ExitStack

import concourse.bass as bass
import concourse.tile as tile
from concourse import bacc, mybir
from concourse import bass_utils
from concourse import tile_utils

alu = mybir.AluOpType
actf = mybir.ActivationFunctionType
F32 = mybir.dt.float32
BF16 = mybir.dt.bfloat16
FP8 = mybir.dt.float8e4

B, C, H, W = 16, 128, 128, 128
H2, W2 = H // 2, W // 2
L = H2 * W2            # 4096
NCORES = 8
BPC = B // NCORES      # 2 images per core
EPS = 1e-5
INV_SQRT_C = 1.0 / np.sqrt(128.0)

# ---- tuning knobs ----
NR_DVE = int(os.environ.get("NR_DVE", "8"))  # conv rows [0,NR_DVE) on DVE
SBUF_CAP = 204 * 1024

KSTAGE = int(os.environ.get("KSTAGE", "9"))

KH_LO = [2, 1, 1, 0, 0, 0, 0]
KH_HI = [64, 64, 64, 64, 64, 63, 63]

TAPS = [(kh, kw) for kh in range(7) for kw in range(7)]


def build(nc):
    x_d = nc.dram_tensor("x", (BPC, C, H, W), BF16, kind="ExternalInput").ap()
    bns_d = nc.dram_tensor("bns", (C, 1), F32, kind="ExternalInput").ap()
    bnb_d = nc.dram_tensor("bnb", (C, 1), F32, kind="ExternalInput").ap()
    dww_d = nc.dram_tensor("dww", (C, 49), F32, kind="ExternalInput").ap()
    dwb_d = nc.dram_tensor("dwb", (C, 1), F32, kind="ExternalInput").ap()
    diag_d = nc.dram_tensor("diag", (C, 49 * 128), FP8,
                            kind="ExternalInput").ap()
    csc_d = nc.dram_tensor("cscale", (C, 1), F32, kind="ExternalInput").ap()
    wqT_d = nc.dram_tensor("wqT", (C, C), BF16, kind="ExternalInput").ap()
    wkT_d = nc.dram_tensor("wkT", (C, C), BF16, kind="ExternalInput").ap()
    wv_d = nc.dram_tensor("wv", (C, C), BF16, kind="ExternalInput").ap()
    pwT_d = nc.dram_tensor("pwT", (C, C), F32, kind="ExternalInput").ap()
    bq_d = nc.dram_tensor("bq", (C, 1), F32, kind="ExternalInput").ap()
    bk_d = nc.dram_tensor("bk", (C, 1), F32, kind="ExternalInput").ap()
    cb_d = nc.dram_tensor("cb", (C, 1), F32, kind="ExternalInput").ap()
    out_d = nc.dram_tensor("out", (BPC, C, H2, W2), F32,
                           kind="ExternalOutput").ap()

    with tile.TileContext(nc) as tc, ExitStack() as ctx:
        tp = lambda name, bufs, **kw: ctx.enter_context(
            tc.tile_pool(name=name, bufs=bufs, **kw))

        wpool = tp("weights", 1)       # persistent small weights
        xp = tp("x", 2)
        gp = tp("gelu", 2)
        t0p = tp("t0", 2)
        q0p = tp("q0", 2)
        accp = tp("acc", 2)
        outp = tp("outc", 3)
        vecp = tp("vec", 16)
        emp = tp("em", 2)
        scrp = tp("scr", 2)
        vtp = tp("vt", 10)
        pp512 = tp("pp512", 2, space="PSUM")
        ppk = tp("ppk", 3, space="PSUM")   # [128,1024] = 2 banks each

        # ---------- persistent weights ----------
        def vec_load(src_ap):
            t = vecp.tile([C, 1], F32, tag="v")
            nc.gpsimd.dma_start(t[:], src_ap)
            return t

        bns = vec_load(bns_d)
        bnb = vec_load(bnb_d)
        dwb = vec_load(dwb_d)
        csc = vec_load(csc_d)
        bq = vec_load(bq_d)
        bk = vec_load(bk_d)
        cb = vec_load(cb_d)

        dww = wpool.tile([C, 49], F32)
        nc.gpsimd.dma_start(dww[:], dww_d)
        diag = wpool.tile([C, 49 * 128], FP8)
        wqT = wpool.tile([C, C], BF16)
        nc.gpsimd.dma_start(wqT[:], wqT_d)
        wkT = wpool.tile([C, C], BF16)
        nc.gpsimd.dma_start(wkT[:], wkT_d)
        wv = wpool.tile([C, C], BF16)
        nc.gpsimd.dma_start(wv[:], wv_d)
        pwT = wpool.tile([C, C], F32)
        nc.gpsimd.dma_start(pwT[:], pwT_d)

        # eo buffers allocated up-front; pads are memset once and never
        # overwritten (gelu writes only the interior), so the per-image
        # gelu never waits on a pad memset
        eo_bufs = []
        for _ in range(BPC):
            eo = gp.tile([C, 2, 134, 68], FP8, tag="eo")
            nc.vector.memset(eo[:, :, 0:3], 0.0)
            nc.vector.memset(eo[:, :, 131:134], 0.0)
            nc.vector.memset(eo[:, 0, 3:131, 0:1], 0.0)
            nc.vector.memset(eo[:, 0, 3:131, 65:68], 0.0)
            nc.vector.memset(eo[:, 1, 3:131, 0:2], 0.0)
            nc.vector.memset(eo[:, 1, 3:131, 66:68], 0.0)
            eo_bufs.append(eo)

        # ---------- per image, software-pipelined emission ----------
        # head(img): dma + gelu + conv + q0 + woven k-dots
        # tail(img): remaining k-dots + softmax + v/proj + out
        # Emitting head(0), head(1), tail(0), tail(1) keeps img1's conv
        # off the critical path of img0's dots/v-phase on every engine.
        def image_head(img):
            x_t = xp.tile([C, H * W], BF16)
            xi = x_d[img].rearrange("c h w -> c (h w)")
            # finer leading slices so gelu band 0 starts ASAP (per-queue
            # DMA bandwidth bounds each slice's completion time)
            xsl = [512 * s for s in range(8)] + \
                  [4096 + 1024 * s for s in range(12)] + [H * W]
            for si, (a, b) in enumerate(zip(xsl, xsl[1:])):
                nc.sync.dma_start(x_t[:, a:b], xi[:, a:b])
                if img == 0 and si == 3:
                    # weights arrive behind the first gelu bands' x slices
                    for s in range(8):
                        sl = slice(s * 784, (s + 1) * 784)
                        nc.sync.dma_start(diag[:, sl], diag_d[:, sl])
            x3 = x_t[:].rearrange("c (h w) -> c h w", h=H)

            # BN+GELU into zero-padded even/odd column buffers:
            #   E[3+r, 1+j] = g[r, 2j]   (width 68, cols 0 and 65.. are pad)
            #   O[3+r, 2+j] = g[r, 2j+1] (cols 0,1 and 66.. are pad)
            # rows 0..2 and 131..133 are pad.
            eo = eo_bufs[img]
            for bnd in range(8):  # fine row bands so conv can start early
                r0, r1 = 16 * bnd, 16 * (bnd + 1)
                nc.scalar.activation(
                    eo[:, 0, 3 + r0:3 + r1, 1:65],
                    x3[:, r0:r1, 0::2], actf.Gelu,
                    bias=bnb[:, 0:1], scale=bns[:, 0:1])
                nc.scalar.activation(
                    eo[:, 1, 3 + r0:3 + r1, 2:66],
                    x3[:, r0:r1, 1::2], actf.Gelu,
                    bias=bnb[:, 0:1], scale=bns[:, 0:1])

            oi = out_d[img].rearrange("c h w -> c (h w)")
            t0 = t0p.tile([C, L], BF16)
            q0 = q0p.tile([C, L], BF16)
            q0sums = vecp.tile([C, 8], F32, tag="q0s")

            def g_ap(kh, kw, a, b):
                # full-rect tap read for h2 in [a,b), all w2: row 2*h2+kh,
                # col (pad+u)+w2 in the parity buffer
                e = kw - 3
                par, u = (0, e // 2) if e % 2 == 0 else (1, (e - 1) // 2)
                off = (1 + u) if par == 0 else (2 + u)
                return eo[:, par, kh + 2 * a:kh + 2 * b:2, off:off + 64]

            def diag_ap(kh, kw):
                t = kh * 7 + kw
                return diag[:, t * 128:(t + 1) * 128]

            def q0_mm(ch):
                ps = pp512.tile([C, 512], F32)
                nc.tensor.matmul(ps[:], wqT[:],
                                 t0[:, ch * 512:(ch + 1) * 512],
                                 start=True, stop=True)
                nc.scalar.activation(q0[:, ch * 512:(ch + 1) * 512], ps[:],
                                     actf.Identity, bias=bq[:, 0:1],
                                     accum_out=q0sums[:, ch:ch + 1])

            # --- conv: DVE part (rows [0, NR_DVE)) — two independent
            # accumulation chains so back-to-back FMAs pipeline without
            # waiting on the previous op's writeback ack ---
            if NR_DVE:
                acc0 = accp.tile([C, NR_DVE, 64], F32, tag="a0")
                acc1 = accp.tile([C, NR_DVE, 64], F32, tag="a1")
                accs = [acc0, acc1]
                for i, (kh, kw) in enumerate(TAPS):
                    w_s = dww[:, kh * 7 + kw:kh * 7 + kw + 1]
                    acc = accs[i % 2]
                    if i < 2:
                        nc.vector.tensor_scalar_mul(
                            acc[:], g_ap(kh, kw, 0, NR_DVE), w_s)
                    else:
                        nc.vector.scalar_tensor_tensor(
                            acc[:], g_ap(kh, kw, 0, NR_DVE), w_s, acc[:],
                            alu.mult, alu.add)
                nc.vector.tensor_tensor(accs[0][:], accs[0][:], accs[1][:],
                                        alu.add)
                nc.scalar.activation(t0[:, :NR_DVE * 64],
                                     accs[0][:].rearrange("c h w -> c (h w)"),
                                     actf.Identity, bias=dwb[:, 0:1])

            # token access patterns (m=0 guide, m>=1 raw-x 2x2 windows)
            def tok_ap(m, c0, c1):
                if m == 0:
                    return t0[:, c0:c1]
                p, q = (m - 1) // 2, (m - 1) % 2
                assert c0 % 64 == 0 and c1 % 64 == 0
                return x3[:, p::2, q::2][:, c0 // 64:c1 // 64, :]

            # dots[:, m*4+hf] = <q0_hf, k_m_hf>; summed in softmax reduce
            dots = vecp.tile([C, 20], F32, tag="dots")

            def k_mm(m, hf):
                # one [C,1024] dot op per (m, ch-pair): 2 matmuls + 1 stt
                kp = ppk.tile([C, 1024], F32, tag="kp")
                for j in range(2):
                    c0 = hf * 1024 + j * 512
                    nc.tensor.matmul(kp[:, j * 512:(j + 1) * 512], wkT[:],
                                     tok_ap(m, c0, c0 + 512),
                                     start=True, stop=True)
                scr = scrp.tile([C, 1024], F32, tag="s")
                nc.vector.scalar_tensor_tensor(
                    scr[:], q0[:, hf * 1024:(hf + 1) * 1024], 1.0, kp[:],
                    alu.mult, alu.mult,
                    accum_out=dots[:, m * 4 + hf:m * 4 + hf + 1])

            # --- conv: PE part (h2 rows [NR_DVE, 64)), fp8 DoubleRow.
            # The even/odd parity planes of eo let tap pairs (kw=2j+1,
            # kw=2j) share one rhs AP with the parity axis as the pair dim:
            # out = diagE.T @ eo[:,0,...] + diagO.T @ eo[:,1,...].
            # 21 pairs + 7 singles (kw=6) instead of 49 matmuls per chunk.
            chunks = [(r, min(r + 8, 64)) for r in range(NR_DVE, 64, 8)]
            for ci, (r0, r1) in enumerate(chunks):
                nrw = r1 - r0
                ps = pp512.tile([C, nrw * 64], F32)
                kq = {  # weave (m, hf) dots; q0 chs land at end of iter ch
                    4: [(0, 1), (1, 1), (2, 1)],
                    5: [(3, 1), (4, 1), (0, 0), (1, 0)],
                    6: [(0, 2), (1, 2), (2, 2), (2, 0), (3, 2), (4, 2)],
                }.get(ci, [])
                for p in range(21):  # pair index: kh = p // 3, j = p % 3
                    kh, j = p // 3, p % 3
                    lhsT = diag[:, p * 256:(p + 1) * 256].rearrange(
                        "c (two f) -> c two f", two=2)
                    rhs = eo[:, :, kh + 2 * r0:kh + 2 * r1:2, j:j + 64]
                    nc.tensor.matmul(ps[:], lhsT, rhs, start=(p == 0),
                                     stop=False,
                                     perf_mode=mybir.MatmulPerfMode.DoubleRow)
                    if p in (3, 7, 11, 15, 19) and kq:
                        k_mm(*kq.pop(0))  # smooth kp flow into DVE
                for pi, kh in enumerate((0, 1, 4)):
                    # kw=6 taps paired (kh, kh+2) via an overlapping-stride
                    # pair dim hand-built on the row axis of the odd plane
                    lhsT = diag[:, (42 + 2 * pi) * 128:
                                (44 + 2 * pi) * 128].rearrange(
                        "c (two f) -> c two f", two=2)
                    rhs = eo[:, 1, kh + 2 * r0:kh + 2 * r1:2, 3:67]
                    rhs = rhs.unsqueeze(1)
                    rhs.ap[1] = [136, 2]  # second plane = rows +2 (tap kh+2)
                    nc.tensor.matmul(ps[:], lhsT, rhs, start=False,
                                     stop=False,
                                     perf_mode=mybir.MatmulPerfMode.DoubleRow)
                nc.tensor.matmul(  # lone tap (kh=5, kw=6)
                    ps[:], diag[:, 48 * 128:49 * 128],
                    eo[:, 1, 5 + 2 * r0:5 + 2 * r1:2, 3:67],
                    start=False, stop=True)
                while kq:
                    k_mm(*kq.pop(0))
                nc.scalar.activation(t0[:, r0 * 64:r1 * 64], ps[:],
                                     actf.Identity, bias=dwb[:, 0:1],
                                     scale=csc[:, 0:1])
                if ci >= 1:
                    q0_mm(ci)  # q0 over the t0 chunk copied last iteration
                if ci == 4:
                    q0_mm(0)  # DVE-region t0 landed during early chunks
            q0_mm(7)
            return dict(oi=oi, dots=dots, q0sums=q0sums, k_mm=k_mm,
                        tok_ap=tok_ap)

        def image_tail(st):
            oi, dots, q0sums = st["oi"], st["dots"], st["q0sums"]
            k_mm, tok_ap = st["k_mm"], st["tok_ap"]
            for mh in ((3, 0), (0, 3), (4, 0),
                       (1, 3), (2, 3), (3, 3), (4, 3)):
                k_mm(*mh)

            # --- softmax over 5 logits (no max-sub: logits are O(1)) ---
            s5 = vecp.tile([C, 5], F32, tag="s5")
            nc.vector.tensor_reduce(
                s5[:], dots[:].rearrange("c (m h) -> c m h", m=5),
                mybir.AxisListType.X, alu.add)
            q0s = vecp.tile([C, 1], F32, tag="v")
            nc.vector.tensor_reduce(q0s[:], q0sums[:], mybir.AxisListType.X,
                                    alu.add)
            bkqs = vecp.tile([C, 1], F32, tag="v")
            nc.vector.scalar_tensor_tensor(bkqs[:], bk[:], INV_SQRT_C, q0s[:],
                                           alu.mult, alu.mult)
            e5 = vecp.tile([C, 5], F32, tag="s5")
            nc.scalar.activation(e5[:], s5[:], actf.Exp, bias=bkqs[:, 0:1],
                                 scale=INV_SQRT_C)
            ssum = vecp.tile([C, 1], F32, tag="v")
            nc.vector.tensor_reduce(ssum[:], e5[:], mybir.AxisListType.X,
                                    alu.add)
            sinv = vecp.tile([C, 1], F32, tag="v")
            nc.vector.reciprocal(sinv[:], ssum[:])
            a5 = vecp.tile([C, 5], F32, tag="s5")
            nc.vector.tensor_scalar_mul(a5[:], e5[:], sinv[:, 0:1])

            # --- fused v+proj: lhsT_m = Wv^T diag(a_m) Pw^T ---
            # em_all[:, m*128+d] = pwT[:, d] * a5[:, m] in one broadcast op,
            # then two matmuls + two copies produce all five vt blocks
            em_all = emp.tile([C, 5, C], BF16, tag="em")
            nc.vector.tensor_tensor(
                em_all[:],
                pwT[:].rearrange("c (m d) -> c m d", m=1).broadcast_to(
                    (C, 5, C)),
                a5[:].rearrange("c (m o) -> c m o", o=1).broadcast_to(
                    (C, 5, C)),
                alu.mult)
            emf = em_all[:].rearrange("c m d -> c (m d)")
            vt_all = vtp.tile([C, 5 * C], BF16, tag="vt")
            for lo, hi in ((0, 4), (4, 5)):
                vp = ppk.tile([C, 512], F32, tag="kp")
                nc.tensor.matmul(vp[:, :(hi - lo) * C], wv[:],
                                 emf[:, lo * C:hi * C], start=True, stop=True)
                nc.scalar.copy(vt_all[:, lo * C:hi * C],
                               vp[:, :(hi - lo) * C])

            for ch in range(8):
                ps = pp512.tile([C, 512], F32)
                for m in range(5):
                    nc.tensor.matmul(
                        ps[:], vt_all[:, m * C:(m + 1) * C],
                        tok_ap(m, ch * 512, (ch + 1) * 512),
                        start=(m == 0), stop=(m == 4))
                oc = outp.tile([C, 512], F32, tag="oc")
                nc.scalar.activation(oc[:], ps[:], actf.Identity,
                                     bias=cb[:, 0:1])
                nc.sync.dma_start(oi[:, ch * 512:(ch + 1) * 512], oc[:])

        states = [image_head(img) for img in range(BPC)]
        for st in states:
            image_tail(st)
    return nc


_CACHE = {}


def _get_nc():
    if "nc" not in _CACHE:
        tile_utils.max_sbuf_usage = SBUF_CAP
        nc = bacc.Bacc("TRN2", target_bir_lowering=False, debug=False,
                       num_devices=NCORES)
        build(nc)
        nc.compile()
        _CACHE["nc"] = nc
    return _CACHE["nc"]


def _in_maps(x, bn_gamma, bn_beta, bn_mean, bn_var, dw_w, dw_b, qkv_w, qkv_b,
             proj_w, proj_b):
    import ml_dtypes
    bf16 = ml_dtypes.bfloat16
    f32 = np.float32
    bn_gamma = np.asarray(bn_gamma, f32)
    bn_beta = np.asarray(bn_beta, f32)
    bn_mean = np.asarray(bn_mean, f32)
    bn_var = np.asarray(bn_var, f32)
    dw_w = np.asarray(dw_w, f32).reshape(C, 49)
    dw_b = np.asarray(dw_b, f32)
    qkv_w = np.asarray(qkv_w, f32)
    qkv_b = np.asarray(qkv_b, f32)
    proj_w = np.asarray(proj_w, f32)
    proj_b = np.asarray(proj_b, f32)

    bns = bn_gamma / np.sqrt(bn_var + np.float32(EPS))
    bnb = bn_beta - bn_mean * bns

    # fp8 conv weights, pre-scaled per channel by a power of two so they
    # sit in e4m3's normal range; the t0 copy descales via ACT's scale.
    wmax = np.abs(dw_w).max(axis=1)
    S = np.exp2(np.floor(np.log2(128.0 / np.maximum(wmax, 1e-30)))).astype(f32)
    cscale = (1.0 / S).reshape(C, 1)
    # block order: 21 pairs [(kh, 2j+1) even-plane, (kh, 2j) odd-plane],
    # then kw=6 row-pairs [(kh,6),(kh+2,6)] for kh in (0,1,4), then (5,6)
    diag = np.zeros((C, 49 * 128), f32)
    blocks = []
    for kh in range(7):
        for j in range(3):
            blocks.append((kh, 2 * j + 1))
            blocks.append((kh, 2 * j))
    for kh in (0, 1, 4):
        blocks.append((kh, 6))
        blocks.append((kh + 2, 6))
    blocks.append((5, 6))
    for bi, (kh, kw) in enumerate(blocks):
        diag[np.arange(C), bi * 128 + np.arange(C)] = dw_w[:, kh * 7 + kw] * S

    bv = qkv_b[2 * C:3 * C]
    fp8 = ml_dtypes.float8_e4m3fn
    shared = {
        "bns": bns.reshape(C, 1),
        "bnb": bnb.reshape(C, 1),
        "dww": np.ascontiguousarray(dw_w),
        "dwb": dw_b.reshape(C, 1),
        "diag": diag.astype(fp8),
        "cscale": cscale,
        "wqT": np.ascontiguousarray(qkv_w[0:C].T).astype(bf16),
        "wkT": np.ascontiguousarray(qkv_w[C:2 * C].T).astype(bf16),
        "wv": np.ascontiguousarray(qkv_w[2 * C:3 * C]).astype(bf16),
        "pwT": np.ascontiguousarray(proj_w.T),
        "bq": qkv_b[0:C].reshape(C, 1),
        "bk": qkv_b[C:2 * C].reshape(C, 1),
        "cb": (proj_w @ bv + proj_b).reshape(C, 1),
    }
    xf = np.ascontiguousarray(np.asarray(x, f32)).astype(bf16)
    return [dict(shared, x=xf[i * BPC:(i + 1) * BPC]) for i in range(NCORES)]


def kernel(**inputs):
    nc = _get_nc()
    res = bass_utils.run_bass_kernel_spmd(nc, _in_maps(**inputs),
                                          core_ids=list(range(NCORES)))
    return np.concatenate([r["out"] for r in res.results], axis=0)
